# revision 16
# baseline (speedup 1.0000x reference)
"""CrossViewSwapAttention Trainium2 kernel (v2 — pipelined rewrite).

Problem (per full input):
  q (1,6,8,8,16,16,128), k/v (1,6,8,8,6,6,128), skip (1,8,8,16,16,128).
  Per window (x,y) of the 8x8 grid: LayerNorm+Linear projections of q/k/v
  tokens, 4-head attention (1536 queries x 216 keys, head dim 32), output
  projection, mean over the 6 views, plus skip.

Sharding: grid x axis (8) across the 8 NeuronCores; each core handles one
row of 8 windows. Weights replicated.

v2 design notes (vs v1 baseline at 379us):
  - Whole-window tiles and 2-deep pools so the Tile scheduler can overlap
    windows; PSUM budget = exactly 8 banks:
      dps x2 (2 banks each) | av | den | zps | prep
  - exp at [108, 2x512] granularity (12 ACT ops/window) straight out of
    PSUM; k-projection pre-scaled by 1/sqrt(dh) so exp uses scale=1.
  - k-side bias dropped entirely (softmax shift invariance); v-side bias
    folded into the output-projection bias (sum att = 1); q-side bias and
    LN gamma folded into weights/bias columns as in v1.
  - scores: per (block, head) one 2-bank PSUM tile [108, 2, 512]; heads
    issued round-robin so row-tiled (K=32, tile_position=(32h,0)) matmuls
    overlap in the PE array.
  - den via ones-matmul (M=32 col bands): replicates the denominator
    across each head band so recip + renormalize are single full-width
    DVE ops per block.
  - bn_aggr replaced by a 7-op stats combine on gpsimd reading bn_stats
    even/odd fields directly; PSUM->SBUF copies also on gpsimd.
  - DMA: one instruction per tensor per window, 1KB descriptors for
    q/skip/out (token index mapped (p c), p=token//2).
"""

import numpy as np

import concourse.bass as bass
import concourse.tile as tile
from concourse import mybir
from concourse.bass_utils import run_bass_kernel_spmd
from concourse.masks import make_identity

F32 = mybir.dt.float32
BF16 = mybir.dt.bfloat16
AF = mybir.ActivationFunctionType
OP = mybir.AluOpType

HEADS = 4
DIM_HEAD = 32
D = 128
NWIN = 8
NVIEW = 6
QTOK = NVIEW * 256        # 1536
KTOK = NVIEW * 36         # 216
KCH = 108                 # keys per chunk (2 chunks)
QB = 512                  # q block (3 blocks per window)
NBLK = QTOK // QB
SCALE = DIM_HEAD ** -0.5
EPS = 1e-5

MAXW = 1  # walrus in this container rejects >1 sync-wait per instruction


def _split_waits(nc, maxw=MAXW):
    """Split multi-sem waits onto same-engine Drain instructions inserted
    immediately before the owning instruction (engine-order equivalent)."""
    for f in nc.m.functions:
        for bb in f.blocks:
            insts = list(bb.instructions)
            newl, changed = [], False
            for inst in insts:
                si = inst.sync_info
                if si is not None and len(si.on_wait) > maxw:
                    waits = list(si.on_wait)
                    changed = True
                    k = 0
                    while len(waits) > maxw:
                        chunk, waits = waits[:maxw], waits[maxw:]
                        newl.append(mybir.InstDrain(
                            name=f"{inst.name}-wsplit{k}",
                            engine=inst.engine,
                            sync_info=mybir.SyncInfo(on_wait=chunk, on_update=[]),
                        ))
                        k += 1
                    inst.sync_info = mybir.SyncInfo(
                        on_wait=waits, on_update=list(si.on_update))
                newl.append(inst)
            if changed:
                bb.instructions = newl


def build_nc():
    nc = bass.Bass()

    q_t = nc.dram_tensor("q", (NVIEW, NWIN, 16, 16, D), F32, kind="ExternalInput")
    k_t = nc.dram_tensor("k", (NVIEW, NWIN, 6, 6, D), F32, kind="ExternalInput")
    v_t = nc.dram_tensor("v", (NVIEW, NWIN, 6, 6, D), F32, kind="ExternalInput")
    skip_t = nc.dram_tensor("skip", (NWIN, 16, 16, D), F32, kind="ExternalInput")
    w_t = nc.dram_tensor("wstack", (4, D, D), F32, kind="ExternalInput")
    p_t = nc.dram_tensor("pstack", (D, 10), F32, kind="ExternalInput")
    out_t = nc.dram_tensor("out", (NWIN, 16, 16, D), F32, kind="ExternalOutput")

    from contextlib import ExitStack
    with tile.TileContext(nc) as tc, ExitStack() as ctx:
        cpool = ctx.enter_context(tc.tile_pool(name="consts", bufs=1))
        sb = ctx.enter_context(tc.tile_pool(name="sb", bufs=2))
        etp = ctx.enter_context(tc.tile_pool(name="et", bufs=2))
        # PSUM: dps(2 banks)x2 + av + den + zps + prep = 8 banks
        dotp = ctx.enter_context(tc.tile_pool(name="dot", bufs=2, space="PSUM"))
        avp = ctx.enter_context(tc.tile_pool(name="avp", bufs=1, space="PSUM"))
        denp = ctx.enter_context(tc.tile_pool(name="denp", bufs=1, space="PSUM"))
        zpsp = ctx.enter_context(tc.tile_pool(name="zpsp", bufs=1, space="PSUM"))
        prep = ctx.enter_context(tc.tile_pool(name="prep", bufs=1, space="PSUM"))

        # ---------------- constants / weight prep ----------------
        wraw = cpool.tile([D, 4, D], F32)
        nc.sync.dma_start(out=wraw, in_=w_t.rearrange("i d o -> d i o"))
        ptile = cpool.tile([D, 10], F32)
        nc.sync.dma_start(out=ptile, in_=p_t[:, :])

        wq_b = cpool.tile([D, D], BF16)
        wk_b = cpool.tile([D, D], BF16)
        wv_b = cpool.tile([D, D], BF16)
        wp_b = cpool.tile([D, D], BF16)
        nc.vector.tensor_scalar_mul(out=wq_b, in0=wraw[:, 0, :], scalar1=ptile[:, 0:1])
        nc.vector.tensor_scalar(out=wk_b, in0=wraw[:, 1, :],
                                scalar1=ptile[:, 2:3], scalar2=SCALE,
                                op0=OP.mult, op1=OP.mult)
        nc.vector.tensor_scalar_mul(out=wv_b, in0=wraw[:, 2, :], scalar1=ptile[:, 4:5])
        nc.vector.tensor_copy(wp_b, wraw[:, 3, :])

        # bwq = Wq^T bq_ln + bq ; bwv = Wv^T bv_ln + bv ; bpe = bp + Wp^T bwv
        bwq = cpool.tile([D, 1], F32)
        bwv = cpool.tile([D, 1], F32)
        bpe = cpool.tile([D, 1], F32)
        bps = prep.tile([D, 512], F32, tag="prep")
        nc.tensor.matmul(bps[:, 0:1], wraw[:, 0, :], ptile[:, 1:2])
        nc.tensor.matmul(bps[:, 1:2], wraw[:, 2, :], ptile[:, 5:6])
        nc.vector.tensor_add(out=bwq, in0=bps[:, 0:1], in1=ptile[:, 6:7])
        nc.vector.tensor_add(out=bwv, in0=bps[:, 1:2], in1=ptile[:, 8:9])
        bps2 = prep.tile([D, 512], F32, tag="prep")
        nc.tensor.matmul(bps2[:, 0:1], wraw[:, 3, :], bwv[:, 0:1])
        nc.vector.tensor_add(out=bpe, in0=bps2[:, 0:1], in1=ptile[:, 9:10])

        id_f32 = cpool.tile([D, D], F32)
        make_identity(nc, id_f32)
        ones_bf = cpool.tile([KCH, DIM_HEAD], BF16)
        nc.vector.memset(ones_bf, 1.0)
        eps_c = cpool.tile([D, 1], F32)
        nc.vector.memset(eps_c, EPS)

        # ---------------- per-window pipeline ----------------
        for w in range(NWIN):
            # ---- loads (token index = 2p + c within each view)
            xq = sb.tile([D, NVIEW, 2, D], F32, tag="xq")
            nc.sync.dma_start(
                out=xq,
                in_=q_t[:, w].rearrange("n a b d -> (a b) n d")
                             .rearrange("(p c) n d -> p n (c d)", c=2))
            xk = sb.tile([KCH, 2, D], F32, tag="xk")
            xv = sb.tile([KCH, 2, D], F32, tag="xv")
            for c in range(2):
                nc.sync.dma_start(
                    out=xk[:, c, :],
                    in_=k_t[3 * c:3 * c + 3, w]
                        .rearrange("n a b d -> n (a b) d"))
                nc.sync.dma_start(
                    out=xv[:, c, :],
                    in_=v_t[3 * c:3 * c + 3, w]
                        .rearrange("n a b d -> n (a b) d"))

            # ---- LN stats: groups 0-11 q (n,c), 12-13 k (c), 14-15 v (c)
            st = sb.tile([D, 16, 6], F32, tag="st")
            nc.gpsimd.memset(st[96:, 12:16, :], 1.0)
            for n in range(NVIEW):
                for c in range(2):
                    nc.vector.bn_stats(out=st[:, 2 * n + c, :],
                                       in_=xq[:, n, c, :])
            for c in range(2):
                nc.vector.bn_stats(out=st[:KCH, 12 + c, :], in_=xk[:, c, :])
                nc.vector.bn_stats(out=st[:KCH, 14 + c, :], in_=xv[:, c, :])

            # stats combine on gpsimd (bn_stats gives even/odd halves):
            #  mu = (m_e + m_o)/2 ; var = (v_e + v_o)/128 + (m_e - m_o)^2/4
            # rs = (var+eps)^-1/2 via Ln/Exp with var4 = v_s/32 + d^2,
            # var = var4/4 (Ln scale=0.25).
            sh = sb.tile([D, 16], F32, tag="sh")    # mu
            vs = sb.tile([D, 16], F32, tag="vs")
            dm = sb.tile([D, 16], F32, tag="dm")
            dd = sb.tile([D, 16], F32, tag="dd")
            t32 = sb.tile([D, 16], F32, tag="t32")
            var4 = sb.tile([D, 16], F32, tag="var4")
            nc.gpsimd.tensor_tensor(out=vs, in0=st[:, :, 2], in1=st[:, :, 5], op=OP.add)
            nc.gpsimd.tensor_tensor(out=dm, in0=st[:, :, 1], in1=st[:, :, 4],
                                    op=OP.subtract)
            nc.gpsimd.tensor_tensor(out=dd, in0=dm, in1=dm, op=OP.mult)
            nc.gpsimd.tensor_scalar(out=t32, in0=vs, scalar1=1.0 / 32.0, scalar2=None,
                                    op0=OP.mult)
            nc.gpsimd.tensor_tensor(out=var4, in0=t32, in1=dd, op=OP.add)
            nc.gpsimd.tensor_tensor(out=sh, in0=st[:, :, 1], in1=st[:, :, 4],
                                    op=OP.add)
            nc.gpsimd.tensor_scalar(out=sh, in0=sh, scalar1=0.5, scalar2=None,
                                    op0=OP.mult)

            lnv = sb.tile([D, 16], F32, tag="lnv")
            rs = sb.tile([D, 16], F32, tag="rs")
            nc.scalar.activation(out=lnv, in_=var4, func=AF.Ln,
                                 bias=eps_c[:, 0:1], scale=0.25)
            nc.scalar.activation(out=rs, in_=lnv, func=AF.Exp, scale=-0.5)

            # ---- normalize -> bf16 (all on gpsimd; k/v padded to 128 rows
            # so the DMA transposes below are clean 128x128 tiles)
            xh_q = sb.tile([D, NVIEW, 2, D], BF16, tag="xhq")
            for n in range(NVIEW):
                for c in range(2):
                    j = 2 * n + c
                    nc.gpsimd.tensor_scalar(
                        out=xh_q[:, n, c, :], in0=xq[:, n, c, :],
                        scalar1=sh[:, j:j + 1], scalar2=rs[:, j:j + 1],
                        op0=OP.subtract, op1=OP.mult)
            xh_k = sb.tile([D, 2, D], BF16, tag="xhk")
            xh_v = sb.tile([D, 2, D], BF16, tag="xhv")
            nc.gpsimd.memset(xh_k[96:, :, :], 0.0)
            nc.gpsimd.memset(xh_v[96:, :, :], 0.0)
            for c in range(2):
                nc.gpsimd.tensor_scalar(
                    out=xh_k[:KCH, c, :], in0=xk[:, c, :],
                    scalar1=sh[:KCH, 12 + c:13 + c], scalar2=rs[:KCH, 12 + c:13 + c],
                    op0=OP.subtract, op1=OP.mult)
                nc.gpsimd.tensor_scalar(
                    out=xh_v[:KCH, c, :], in0=xv[:, c, :],
                    scalar1=sh[:KCH, 14 + c:15 + c], scalar2=rs[:KCH, 14 + c:15 + c],
                    op0=OP.subtract, op1=OP.mult)

            # ---- transposes to feature-major via DMA xbar (SBUF->SBUF)
            xhqT = sb.tile([D, QTOK], BF16, tag="xhqT")
            xhkT = sb.tile([D, 2, D], BF16, tag="xhkT")
            xhvT = sb.tile([D, 2, D], BF16, tag="xhvT")
            for j in range(12):
                nc.sync.dma_start_transpose(
                    out=xhqT[:, 128 * j:128 * j + 128], in_=xh_q[:, j // 2, j % 2, :])
            for c in range(2):
                nc.sync.dma_start_transpose(out=xhkT[:, c, :], in_=xh_k[:, c, :])
                nc.sync.dma_start_transpose(out=xhvT[:, c, :], in_=xh_v[:, c, :])

            # ---- projections
            qhT = sb.tile([D, QTOK], BF16, tag="qhT")
            for g in range(3):
                pq = prep.tile([D, 512], F32, tag="prep")
                nc.tensor.matmul(pq, wq_b, xhqT[:, 512 * g:512 * g + 512])
                nc.vector.tensor_scalar(
                    out=qhT[:, 512 * g:512 * g + 512], in0=pq,
                    scalar1=bwq[:, 0:1], scalar2=None, op0=OP.add)
            khT = sb.tile([D, 2, KCH], BF16, tag="khT")
            vh = sb.tile([KCH, 2, D], BF16, tag="vh")
            pkv = prep.tile([D, 512], F32, tag="prep")
            for c in range(2):
                nc.tensor.matmul(pkv[:, 128 * c:128 * c + KCH], wk_b,
                                 xhkT[:, c, :KCH])
                nc.tensor.matmul(pkv[:KCH, 256 + 128 * c:256 + 128 * c + 128],
                                 xhvT[:, c, :KCH], wv_b)
            nc.vector.tensor_copy(
                khT, pkv[:, 0:384].rearrange("p (c k) -> p c k", c=3)[:, 0:2, :KCH])
            nc.vector.tensor_copy(
                vh, pkv[:KCH, 256:512].rearrange("p (a f) -> p a f", a=2))

            # ---- attention: per block, per head: scores -> exp; then av/den
            ets = []
            aT = sb.tile([D, QTOK], BF16, tag="aT")
            zps = zpsp.tile([D, 512], F32, tag="zps")
            for hd in range(HEADS):
                et = etp.tile([KCH, 2, QTOK], BF16, tag=f"et{hd}")
                ets.append(et)
            for b in range(NBLK):
                q0 = QB * b
                for hd in range(HEADS):
                    dps = dotp.tile([KCH, 2, QB], F32, tag="dot")
                    for c in range(2):
                        nc.tensor.matmul(
                            dps[:, c, :],
                            khT[32 * hd:32 * hd + 32, c, :],
                            qhT[32 * hd:32 * hd + 32, q0:q0 + QB],
                            tile_position=(32 * hd, 0))
                    nc.scalar.activation(
                        out=ets[hd][:, :, q0:q0 + QB], in_=dps, func=AF.Exp)
                av = avp.tile([D, QB], F32, tag="av")
                den = denp.tile([D, QB], F32, tag="den")
                for hd in range(HEADS):
                    for c in range(2):
                        nc.tensor.matmul(
                            den[32 * hd:32 * hd + 32, :],
                            ones_bf, ets[hd][:, c, q0:q0 + QB],
                            start=(c == 0), stop=(c == 1),
                            tile_position=(0, 32 * hd))
                        nc.tensor.matmul(
                            av[32 * hd:32 * hd + 32, :],
                            vh[:, c, 32 * hd:32 * hd + 32],
                            ets[hd][:, c, q0:q0 + QB],
                            start=(c == 0), stop=(c == 1),
                            tile_position=(0, 32 * hd))
                recipT = sb.tile([D, QB], F32, tag="recipT")
                nc.vector.reciprocal(out=recipT, in_=den)
                nc.vector.tensor_tensor(
                    out=aT[:, q0:q0 + QB], in0=av, in1=recipT, op=OP.mult)
                # out-projection: accumulate the two views of this block
                for u in range(2):
                    n = 2 * b + u
                    nc.tensor.matmul(zps[:, 0:256], wp_b,
                                     aT[:, 256 * n:256 * n + 256],
                                     start=(n == 0), stop=(n == NVIEW - 1))

            # ---- epilogue: mean+bias, transpose back, skip, store
            outT = sb.tile([D, 256], F32, tag="outT")
            nc.vector.tensor_scalar(
                out=outT, in0=zps[:, 0:256], scalar1=1.0 / NVIEW,
                scalar2=bpe[:, 0:1], op0=OP.mult, op1=OP.add)
            sk = sb.tile([D, 2, D], F32, tag="sk")
            nc.sync.dma_start(
                out=sk,
                in_=skip_t[w].rearrange("a b d -> (a b) d")
                             .rearrange("(p c) d -> p (c d)", c=2))
            fps = prep.tile([D, 512], F32, tag="prep")
            for i in range(2):
                nc.tensor.transpose(fps[:, 128 * i:128 * i + 128],
                                    outT[:, 128 * i:128 * i + 128], id_f32)
            res = sb.tile([D, 2, D], F32, tag="res")
            nc.vector.tensor_tensor(
                out=res, in0=fps[:, 0:256].rearrange("p (c d) -> p c d", c=2),
                in1=sk, op=OP.add)
            nc.sync.dma_start(
                out=out_t[w].rearrange("a b d -> (a b) d")
                            .rearrange("(p c) d -> p (c d)", c=2),
                in_=res)

    _split_waits(nc)
    return nc


_NC_CACHE = None


def _get_nc():
    global _NC_CACHE
    if _NC_CACHE is None:
        _NC_CACHE = build_nc()
    return _NC_CACHE


def kernel(**inputs):
    q = np.asarray(inputs["q"], dtype=np.float32)
    k = np.asarray(inputs["k"], dtype=np.float32)
    v = np.asarray(inputs["v"], dtype=np.float32)
    skip = np.asarray(inputs["skip"], dtype=np.float32)

    wstack = np.stack([inputs["Wq"], inputs["Wk"], inputs["Wv"], inputs["Wp"]]
                      ).astype(np.float32)
    pstack = np.stack([
        inputs["gq"], inputs["bq_ln"], inputs["gk"], inputs["bk_ln"],
        inputs["gv"], inputs["bv_ln"], inputs["bq"], inputs["bk"],
        inputs["bv"], inputs["bp"]], axis=1).astype(np.float32)

    nc = _get_nc()
    in_maps = []
    for c in range(8):
        in_maps.append({
            "q": np.ascontiguousarray(q[0, :, c]),
            "k": np.ascontiguousarray(k[0, :, c]),
            "v": np.ascontiguousarray(v[0, :, c]),
            "skip": np.ascontiguousarray(skip[0, c]),
            "wstack": wstack,
            "pstack": pstack,
        })
    import os
    trace = bool(os.environ.get("KERNEL_TRACE"))
    res = run_bass_kernel_spmd(nc, in_maps, core_ids=list(range(8)),
                               trace=trace)
    kernel.last_result = res
    out = np.stack([res.results[c]["out"] for c in range(8)], axis=0)
    return out[None]  # (1, 8, 8, 16, 16, 128)


# revision 20
# speedup vs baseline: 1.5455x; 1.5455x over previous
"""CrossViewSwapAttention Trainium2 kernel (v2 — pipelined rewrite).

Problem (per full input):
  q (1,6,8,8,16,16,128), k/v (1,6,8,8,6,6,128), skip (1,8,8,16,16,128).
  Per window (x,y) of the 8x8 grid: LayerNorm+Linear projections of q/k/v
  tokens, 4-head attention (1536 queries x 216 keys, head dim 32), output
  projection, mean over the 6 views, plus skip.

Sharding: grid x axis (8) across the 8 NeuronCores; each core handles one
row of 8 windows. Weights replicated.

v2 design notes (vs v1 baseline at 379us):
  - Whole-window tiles and 2-deep pools so the Tile scheduler can overlap
    windows; PSUM budget = exactly 8 banks:
      dps x2 (2 banks each) | av | den | zps | prep
  - exp at [108, 2x512] granularity (12 ACT ops/window) straight out of
    PSUM; k-projection pre-scaled by 1/sqrt(dh) so exp uses scale=1.
  - k-side bias dropped entirely (softmax shift invariance); v-side bias
    folded into the output-projection bias (sum att = 1); q-side bias and
    LN gamma folded into weights/bias columns as in v1.
  - scores: per (block, head) one 2-bank PSUM tile [108, 2, 512]; heads
    issued round-robin so row-tiled (K=32, tile_position=(32h,0)) matmuls
    overlap in the PE array.
  - den via ones-matmul (M=32 col bands): replicates the denominator
    across each head band so recip + renormalize are single full-width
    DVE ops per block.
  - bn_aggr replaced by a 7-op stats combine on gpsimd reading bn_stats
    even/odd fields directly; PSUM->SBUF copies also on gpsimd.
  - DMA: one instruction per tensor per window, 1KB descriptors for
    q/skip/out (token index mapped (p c), p=token//2).
"""

import numpy as np

import concourse.bass as bass
import concourse.tile as tile
from concourse import mybir
from concourse.bass_utils import run_bass_kernel_spmd
from concourse.masks import make_identity

F32 = mybir.dt.float32
BF16 = mybir.dt.bfloat16
AF = mybir.ActivationFunctionType
OP = mybir.AluOpType

HEADS = 4
DIM_HEAD = 32
D = 128
NWIN = 8
NVIEW = 6
QTOK = NVIEW * 256        # 1536
KTOK = NVIEW * 36         # 216
KCH = 108                 # keys per chunk (2 chunks)
QB = 512                  # q block (3 blocks per window)
NBLK = QTOK // QB
SCALE = DIM_HEAD ** -0.5
EPS = 1e-5

MAXW = 1  # walrus in this container rejects >1 sync-wait per instruction


def _split_waits(nc, maxw=MAXW):
    """Split multi-sem waits onto same-engine Drain instructions inserted
    immediately before the owning instruction (engine-order equivalent)."""
    for f in nc.m.functions:
        for bb in f.blocks:
            insts = list(bb.instructions)
            newl, changed = [], False
            for inst in insts:
                si = inst.sync_info
                if si is not None and len(si.on_wait) > maxw:
                    waits = list(si.on_wait)
                    changed = True
                    k = 0
                    while len(waits) > maxw:
                        chunk, waits = waits[:maxw], waits[maxw:]
                        newl.append(mybir.InstDrain(
                            name=f"{inst.name}-wsplit{k}",
                            engine=inst.engine,
                            sync_info=mybir.SyncInfo(on_wait=chunk, on_update=[]),
                        ))
                        k += 1
                    inst.sync_info = mybir.SyncInfo(
                        on_wait=waits, on_update=list(si.on_update))
                newl.append(inst)
            if changed:
                bb.instructions = newl


def build_nc():
    nc = bass.Bass()

    q_t = nc.dram_tensor("q", (NVIEW, NWIN, 16, 16, D), F32, kind="ExternalInput")
    k_t = nc.dram_tensor("k", (NVIEW, NWIN, 6, 6, D), F32, kind="ExternalInput")
    v_t = nc.dram_tensor("v", (NVIEW, NWIN, 6, 6, D), F32, kind="ExternalInput")
    skip_t = nc.dram_tensor("skip", (NWIN, 16, 16, D), F32, kind="ExternalInput")
    w_t = nc.dram_tensor("wstack", (4, D, D), F32, kind="ExternalInput")
    p_t = nc.dram_tensor("pstack", (D, 10), F32, kind="ExternalInput")
    out_t = nc.dram_tensor("out", (NWIN, 16, 16, D), F32, kind="ExternalOutput")

    from contextlib import ExitStack
    with tile.TileContext(nc) as tc, ExitStack() as ctx:
        cpool = ctx.enter_context(tc.tile_pool(name="consts", bufs=1))
        sb = ctx.enter_context(tc.tile_pool(name="sb", bufs=2))
        etp = ctx.enter_context(tc.tile_pool(name="et", bufs=2))
        # PSUM: dps(2 banks)x2 + av + den + zps + prep = 8 banks
        dotp = ctx.enter_context(tc.tile_pool(name="dot", bufs=2, space="PSUM"))
        avp = ctx.enter_context(tc.tile_pool(name="avp", bufs=1, space="PSUM"))
        denp = ctx.enter_context(tc.tile_pool(name="denp", bufs=1, space="PSUM"))
        zpsp = ctx.enter_context(tc.tile_pool(name="zpsp", bufs=1, space="PSUM"))
        prep = ctx.enter_context(tc.tile_pool(name="prep", bufs=1, space="PSUM"))

        # ---------------- constants / weight prep ----------------
        wraw = cpool.tile([D, 4, D], F32)
        nc.sync.dma_start(out=wraw, in_=w_t.rearrange("i d o -> d i o"))
        ptile = cpool.tile([D, 10], F32)
        nc.sync.dma_start(out=ptile, in_=p_t[:, :])

        wq_b = cpool.tile([D, D], BF16)
        wk_b = cpool.tile([D, D], BF16)
        wv_b = cpool.tile([D, D], BF16)
        wp_b = cpool.tile([D, D], BF16)
        nc.vector.tensor_scalar_mul(out=wq_b, in0=wraw[:, 0, :], scalar1=ptile[:, 0:1])
        nc.vector.tensor_scalar(out=wk_b, in0=wraw[:, 1, :],
                                scalar1=ptile[:, 2:3], scalar2=SCALE,
                                op0=OP.mult, op1=OP.mult)
        nc.vector.tensor_scalar_mul(out=wv_b, in0=wraw[:, 2, :], scalar1=ptile[:, 4:5])
        nc.vector.tensor_copy(wp_b, wraw[:, 3, :])

        # bwq = Wq^T bq_ln + bq ; bwv = Wv^T bv_ln + bv ; bpe = bp + Wp^T bwv
        bwq = cpool.tile([D, 1], F32)
        bwv = cpool.tile([D, 1], F32)
        bpe = cpool.tile([D, 1], F32)
        bps = prep.tile([D, 512], F32, tag="prep")
        nc.tensor.matmul(bps[:, 0:1], wraw[:, 0, :], ptile[:, 1:2])
        nc.tensor.matmul(bps[:, 1:2], wraw[:, 2, :], ptile[:, 5:6])
        nc.vector.tensor_add(out=bwq, in0=bps[:, 0:1], in1=ptile[:, 6:7])
        nc.vector.tensor_add(out=bwv, in0=bps[:, 1:2], in1=ptile[:, 8:9])
        bps2 = prep.tile([D, 512], F32, tag="prep")
        nc.tensor.matmul(bps2[:, 0:1], wraw[:, 3, :], bwv[:, 0:1])
        nc.vector.tensor_add(out=bpe, in0=bps2[:, 0:1], in1=ptile[:, 9:10])

        id_bf = cpool.tile([D, D], BF16)
        id_f32 = cpool.tile([D, D], F32)
        make_identity(nc, id_bf)
        make_identity(nc, id_f32)
        ones_bf = cpool.tile([KCH, DIM_HEAD], BF16)
        nc.vector.memset(ones_bf, 1.0)
        eps_c = cpool.tile([D, 1], F32)
        nc.vector.memset(eps_c, EPS)

        # ---------------- per-window pipeline ----------------
        for w in range(NWIN):
            # ---- loads (token index = 2p + c within each view)
            xq = sb.tile([D, NVIEW, 2, D], F32, tag="xq")
            nc.sync.dma_start(
                out=xq,
                in_=q_t[:, w].rearrange("n a b d -> (a b) n d")
                             .rearrange("(p c) n d -> p n (c d)", c=2))
            xk = sb.tile([KCH, 2, D], F32, tag="xk")
            xv = sb.tile([KCH, 2, D], F32, tag="xv")
            for c in range(2):
                nc.sync.dma_start(
                    out=xk[:, c, :],
                    in_=k_t[3 * c:3 * c + 3, w]
                        .rearrange("n a b d -> n (a b) d"))
                nc.sync.dma_start(
                    out=xv[:, c, :],
                    in_=v_t[3 * c:3 * c + 3, w]
                        .rearrange("n a b d -> n (a b) d"))

            # ---- LN stats: groups 0-11 q (n,c), 12-13 k (c), 14-15 v (c)
            st = sb.tile([D, 16, 6], F32, tag="st")
            nc.gpsimd.memset(st[96:, 12:16, :], 1.0)
            for n in range(NVIEW):
                for c in range(2):
                    nc.vector.bn_stats(out=st[:, 2 * n + c, :],
                                       in_=xq[:, n, c, :])
            for c in range(2):
                nc.vector.bn_stats(out=st[:KCH, 12 + c, :], in_=xk[:, c, :])
                nc.vector.bn_stats(out=st[:KCH, 14 + c, :], in_=xv[:, c, :])

            # stats combine on gpsimd (bn_stats gives even/odd halves):
            #  mu = (m_e + m_o)/2 ; var = (v_e + v_o)/128 + (m_e - m_o)^2/4
            # rs = (var+eps)^-1/2 via Ln/Exp with var4 = v_s/32 + d^2,
            # var = var4/4 (Ln scale=0.25).
            sh = sb.tile([D, 16], F32, tag="sh")    # mu
            vs = sb.tile([D, 16], F32, tag="vs")
            dm = sb.tile([D, 16], F32, tag="dm")
            dd = sb.tile([D, 16], F32, tag="dd")
            t32 = sb.tile([D, 16], F32, tag="t32")
            var4 = sb.tile([D, 16], F32, tag="var4")
            nc.gpsimd.tensor_tensor(out=vs, in0=st[:, :, 2], in1=st[:, :, 5], op=OP.add)
            nc.gpsimd.tensor_tensor(out=dm, in0=st[:, :, 1], in1=st[:, :, 4],
                                    op=OP.subtract)
            nc.gpsimd.tensor_tensor(out=dd, in0=dm, in1=dm, op=OP.mult)
            nc.gpsimd.tensor_scalar(out=t32, in0=vs, scalar1=1.0 / 32.0, scalar2=None,
                                    op0=OP.mult)
            nc.gpsimd.tensor_tensor(out=var4, in0=t32, in1=dd, op=OP.add)
            nc.gpsimd.tensor_tensor(out=sh, in0=st[:, :, 1], in1=st[:, :, 4],
                                    op=OP.add)
            nc.gpsimd.tensor_scalar(out=sh, in0=sh, scalar1=0.5, scalar2=None,
                                    op0=OP.mult)

            lnv = sb.tile([D, 16], F32, tag="lnv")
            rs = sb.tile([D, 16], F32, tag="rs")
            nc.scalar.activation(out=lnv, in_=var4, func=AF.Ln,
                                 bias=eps_c[:, 0:1], scale=0.25)
            nc.scalar.activation(out=rs, in_=lnv, func=AF.Exp, scale=-0.5)

            # ---- normalize -> bf16 (all on gpsimd; k/v padded to 128 rows
            # so the DMA transposes below are clean 128x128 tiles)
            xh_q = sb.tile([D, NVIEW, 2, D], BF16, tag="xhq")
            for n in range(NVIEW):
                for c in range(2):
                    j = 2 * n + c
                    nc.vector.tensor_scalar(
                        out=xh_q[:, n, c, :], in0=xq[:, n, c, :],
                        scalar1=sh[:, j:j + 1], scalar2=rs[:, j:j + 1],
                        op0=OP.subtract, op1=OP.mult)
            xh_k = sb.tile([KCH, 2, D], BF16, tag="xhk")
            xh_v = sb.tile([KCH, 2, D], BF16, tag="xhv")
            for c in range(2):
                nc.vector.tensor_scalar(
                    out=xh_k[:KCH, c, :], in0=xk[:, c, :],
                    scalar1=sh[:KCH, 12 + c:13 + c], scalar2=rs[:KCH, 12 + c:13 + c],
                    op0=OP.subtract, op1=OP.mult)
                nc.vector.tensor_scalar(
                    out=xh_v[:KCH, c, :], in0=xv[:, c, :],
                    scalar1=sh[:KCH, 14 + c:15 + c], scalar2=rs[:KCH, 14 + c:15 + c],
                    op0=OP.subtract, op1=OP.mult)

            # ---- transposes to feature-major: PE, then DMA drains (bf16)
            xhqT = sb.tile([D, QTOK], BF16, tag="xhqT")
            xhkvT = sb.tile([D, 4, KCH], BF16, tag="xhkvT")
            tp1 = prep.tile([D, 1024], BF16, tag="prep")
            for j in range(8):
                nc.tensor.transpose(tp1[:, 128 * j:128 * j + 128],
                                    xh_q[:, j // 2, j % 2, :], id_bf)
            nc.scalar.copy(xhqT[:, 0:1024], tp1)
            tp2 = prep.tile([D, 1024], BF16, tag="prep")
            for j in range(4):
                nc.tensor.transpose(tp2[:, 128 * j:128 * j + 128],
                                    xh_q[:, (8 + j) // 2, j % 2, :], id_bf)
            for c in range(2):
                nc.tensor.transpose(tp2[:, 512 + KCH * c:512 + KCH * c + KCH],
                                    xh_k[:, c, :], id_bf[:KCH, :KCH])
                nc.tensor.transpose(tp2[:, 728 + KCH * c:728 + KCH * c + KCH],
                                    xh_v[:, c, :], id_bf[:KCH, :KCH])
            nc.vector.tensor_copy(xhqT[:, 1024:1536], tp2[:, 0:512])
            nc.vector.tensor_copy(xhkvT.rearrange("p g k -> p (g k)"),
                                  tp2[:, 512:944])

            # ---- projections
            qhT = sb.tile([D, QTOK], BF16, tag="qhT")
            for g in range(3):
                pq = prep.tile([D, 512], F32, tag="prep")
                nc.tensor.matmul(pq, wq_b, xhqT[:, 512 * g:512 * g + 512])
                nc.vector.tensor_scalar(
                    out=qhT[:, 512 * g:512 * g + 512], in0=pq,
                    scalar1=bwq[:, 0:1], scalar2=None, op0=OP.add)
            khT = sb.tile([D, 2, KCH], BF16, tag="khT")
            vh = sb.tile([KCH, 2, D], BF16, tag="vh")
            pkv = prep.tile([D, 512], F32, tag="prep")
            for c in range(2):
                nc.tensor.matmul(pkv[:, 128 * c:128 * c + KCH], wk_b,
                                 xhkvT[:, c, :])
                nc.tensor.matmul(pkv[:KCH, 256 + 128 * c:256 + 128 * c + 128],
                                 xhkvT[:, 2 + c, :], wv_b)
            nc.scalar.copy(
                khT, pkv[:, 0:384].rearrange("p (c k) -> p c k", c=3)[:, 0:2, :KCH])
            nc.scalar.copy(
                vh, pkv[:KCH, 256:512].rearrange("p (a f) -> p a f", a=2))

            # ---- attention: per block, per head: scores -> exp; then av/den
            ets = []
            aT = sb.tile([D, QTOK], BF16, tag="aT")
            zps = zpsp.tile([D, 512], F32, tag="zps")
            for hd in range(HEADS):
                et = etp.tile([KCH, 2, QTOK], BF16, tag=f"et{hd}")
                ets.append(et)
            for b in range(NBLK):
                q0 = QB * b
                for hd in range(HEADS):
                    dps = dotp.tile([KCH, 2, QB], F32, tag="dot")
                    for c in range(2):
                        nc.tensor.matmul(
                            dps[:, c, :],
                            khT[32 * hd:32 * hd + 32, c, :],
                            qhT[32 * hd:32 * hd + 32, q0:q0 + QB],
                            tile_position=(32 * hd, 0))
                    nc.scalar.activation(
                        out=ets[hd][:, :, q0:q0 + QB], in_=dps, func=AF.Exp)
                av = avp.tile([D, QB], F32, tag="av")
                den = denp.tile([D, QB], F32, tag="den")
                for hd in range(HEADS):
                    for c in range(2):
                        nc.tensor.matmul(
                            den[32 * hd:32 * hd + 32, :],
                            ones_bf, ets[hd][:, c, q0:q0 + QB],
                            start=(c == 0), stop=(c == 1),
                            tile_position=(0, 32 * hd))
                        nc.tensor.matmul(
                            av[32 * hd:32 * hd + 32, :],
                            vh[:, c, 32 * hd:32 * hd + 32],
                            ets[hd][:, c, q0:q0 + QB],
                            start=(c == 0), stop=(c == 1),
                            tile_position=(0, 32 * hd))
                rln = sb.tile([D, QB], F32, tag="rln")
                recipT = sb.tile([D, QB], F32, tag="recipT")
                nc.scalar.activation(out=rln, in_=den, func=AF.Ln,
                                     bias=eps_c[:, 0:1])
                nc.scalar.activation(out=recipT, in_=rln, func=AF.Exp,
                                     scale=-1.0)
                nc.vector.tensor_tensor(
                    out=aT[:, q0:q0 + QB], in0=av, in1=recipT, op=OP.mult)
                # out-projection: accumulate the two views of this block
                for u in range(2):
                    n = 2 * b + u
                    nc.tensor.matmul(zps[:, 0:256], wp_b,
                                     aT[:, 256 * n:256 * n + 256],
                                     start=(n == 0), stop=(n == NVIEW - 1))

            # ---- epilogue: mean+bias, transpose back, skip, store
            outT = sb.tile([D, 256], F32, tag="outT")
            nc.vector.tensor_scalar(
                out=outT, in0=zps[:, 0:256], scalar1=1.0 / NVIEW,
                scalar2=bpe[:, 0:1], op0=OP.mult, op1=OP.add)
            sk = sb.tile([D, 2, D], F32, tag="sk")
            nc.sync.dma_start(
                out=sk,
                in_=skip_t[w].rearrange("a b d -> (a b) d")
                             .rearrange("(p c) d -> p (c d)", c=2))
            fps = prep.tile([D, 512], F32, tag="prep")
            for i in range(2):
                nc.tensor.transpose(fps[:, 128 * i:128 * i + 128],
                                    outT[:, 128 * i:128 * i + 128], id_f32)
            res = sb.tile([D, 2, D], F32, tag="res")
            nc.vector.tensor_tensor(
                out=res, in0=fps[:, 0:256].rearrange("p (c d) -> p c d", c=2),
                in1=sk, op=OP.add)
            nc.sync.dma_start(
                out=out_t[w].rearrange("a b d -> (a b) d")
                            .rearrange("(p c) d -> p (c d)", c=2),
                in_=res)

    _split_waits(nc)
    return nc


_NC_CACHE = None


def _get_nc():
    global _NC_CACHE
    if _NC_CACHE is None:
        _NC_CACHE = build_nc()
    return _NC_CACHE


def kernel(**inputs):
    q = np.asarray(inputs["q"], dtype=np.float32)
    k = np.asarray(inputs["k"], dtype=np.float32)
    v = np.asarray(inputs["v"], dtype=np.float32)
    skip = np.asarray(inputs["skip"], dtype=np.float32)

    wstack = np.stack([inputs["Wq"], inputs["Wk"], inputs["Wv"], inputs["Wp"]]
                      ).astype(np.float32)
    pstack = np.stack([
        inputs["gq"], inputs["bq_ln"], inputs["gk"], inputs["bk_ln"],
        inputs["gv"], inputs["bv_ln"], inputs["bq"], inputs["bk"],
        inputs["bv"], inputs["bp"]], axis=1).astype(np.float32)

    nc = _get_nc()
    in_maps = []
    for c in range(8):
        in_maps.append({
            "q": np.ascontiguousarray(q[0, :, c]),
            "k": np.ascontiguousarray(k[0, :, c]),
            "v": np.ascontiguousarray(v[0, :, c]),
            "skip": np.ascontiguousarray(skip[0, c]),
            "wstack": wstack,
            "pstack": pstack,
        })
    import os
    trace = bool(os.environ.get("KERNEL_TRACE"))
    res = run_bass_kernel_spmd(nc, in_maps, core_ids=list(range(8)),
                               trace=trace)
    kernel.last_result = res
    out = np.stack([res.results[c]["out"] for c in range(8)], axis=0)
    return out[None]  # (1, 8, 8, 16, 16, 128)


# revision 21
# speedup vs baseline: 1.5620x; 1.0107x over previous
"""CrossViewSwapAttention Trainium2 kernel (v2 — pipelined rewrite).

Problem (per full input):
  q (1,6,8,8,16,16,128), k/v (1,6,8,8,6,6,128), skip (1,8,8,16,16,128).
  Per window (x,y) of the 8x8 grid: LayerNorm+Linear projections of q/k/v
  tokens, 4-head attention (1536 queries x 216 keys, head dim 32), output
  projection, mean over the 6 views, plus skip.

Sharding: grid x axis (8) across the 8 NeuronCores; each core handles one
row of 8 windows. Weights replicated.

v2 design notes (vs v1 baseline at 379us):
  - Whole-window tiles and 2-deep pools so the Tile scheduler can overlap
    windows; PSUM budget = exactly 8 banks:
      dps x2 (2 banks each) | av | den | zps | prep
  - exp at [108, 2x512] granularity (12 ACT ops/window) straight out of
    PSUM; k-projection pre-scaled by 1/sqrt(dh) so exp uses scale=1.
  - k-side bias dropped entirely (softmax shift invariance); v-side bias
    folded into the output-projection bias (sum att = 1); q-side bias and
    LN gamma folded into weights/bias columns as in v1.
  - scores: per (block, head) one 2-bank PSUM tile [108, 2, 512]; heads
    issued round-robin so row-tiled (K=32, tile_position=(32h,0)) matmuls
    overlap in the PE array.
  - den via ones-matmul (M=32 col bands): replicates the denominator
    across each head band so recip + renormalize are single full-width
    DVE ops per block.
  - bn_aggr replaced by a 7-op stats combine on gpsimd reading bn_stats
    even/odd fields directly; PSUM->SBUF copies also on gpsimd.
  - DMA: one instruction per tensor per window, 1KB descriptors for
    q/skip/out (token index mapped (p c), p=token//2).
"""

import numpy as np

import concourse.bass as bass
import concourse.tile as tile
from concourse import mybir
from concourse.bass_utils import run_bass_kernel_spmd
from concourse.masks import make_identity

F32 = mybir.dt.float32
BF16 = mybir.dt.bfloat16
AF = mybir.ActivationFunctionType
OP = mybir.AluOpType

HEADS = 4
DIM_HEAD = 32
D = 128
NWIN = 8
NVIEW = 6
QTOK = NVIEW * 256        # 1536
KTOK = NVIEW * 36         # 216
KCH = 108                 # keys per chunk (2 chunks)
QB = 512                  # q block (3 blocks per window)
NBLK = QTOK // QB
SCALE = DIM_HEAD ** -0.5
EPS = 1e-5

MAXW = 1  # walrus in this container rejects >1 sync-wait per instruction


def _split_waits(nc, maxw=MAXW):
    """Split multi-sem waits onto same-engine Drain instructions inserted
    immediately before the owning instruction (engine-order equivalent)."""
    for f in nc.m.functions:
        for bb in f.blocks:
            insts = list(bb.instructions)
            newl, changed = [], False
            for inst in insts:
                si = inst.sync_info
                if si is not None and len(si.on_wait) > maxw:
                    waits = list(si.on_wait)
                    changed = True
                    k = 0
                    while len(waits) > maxw:
                        chunk, waits = waits[:maxw], waits[maxw:]
                        newl.append(mybir.InstDrain(
                            name=f"{inst.name}-wsplit{k}",
                            engine=inst.engine,
                            sync_info=mybir.SyncInfo(on_wait=chunk, on_update=[]),
                        ))
                        k += 1
                    inst.sync_info = mybir.SyncInfo(
                        on_wait=waits, on_update=list(si.on_update))
                newl.append(inst)
            if changed:
                bb.instructions = newl


def build_nc():
    nc = bass.Bass()

    q_t = nc.dram_tensor("q", (NVIEW, NWIN, 16, 16, D), F32, kind="ExternalInput")
    k_t = nc.dram_tensor("k", (NVIEW, NWIN, 6, 6, D), F32, kind="ExternalInput")
    v_t = nc.dram_tensor("v", (NVIEW, NWIN, 6, 6, D), F32, kind="ExternalInput")
    skip_t = nc.dram_tensor("skip", (NWIN, 16, 16, D), F32, kind="ExternalInput")
    w_t = nc.dram_tensor("wstack", (4, D, D), F32, kind="ExternalInput")
    p_t = nc.dram_tensor("pstack", (D, 10), F32, kind="ExternalInput")
    out_t = nc.dram_tensor("out", (NWIN, 16, 16, D), F32, kind="ExternalOutput")

    from contextlib import ExitStack
    with tile.TileContext(nc) as tc, ExitStack() as ctx:
        cpool = ctx.enter_context(tc.tile_pool(name="consts", bufs=1))
        sb = ctx.enter_context(tc.tile_pool(name="sb", bufs=2))
        etp = ctx.enter_context(tc.tile_pool(name="et", bufs=2))
        # PSUM: dps(2 banks)x2 + av + den + zps + prep = 8 banks
        dotp = ctx.enter_context(tc.tile_pool(name="dot", bufs=2, space="PSUM"))
        avp = ctx.enter_context(tc.tile_pool(name="avp", bufs=1, space="PSUM"))
        denp = ctx.enter_context(tc.tile_pool(name="denp", bufs=1, space="PSUM"))
        zpsp = ctx.enter_context(tc.tile_pool(name="zpsp", bufs=1, space="PSUM"))
        prep = ctx.enter_context(tc.tile_pool(name="prep", bufs=1, space="PSUM"))

        # ---------------- constants / weight prep ----------------
        wraw = cpool.tile([D, 4, D], F32)
        nc.sync.dma_start(out=wraw, in_=w_t.rearrange("i d o -> d i o"))
        ptile = cpool.tile([D, 10], F32)
        nc.sync.dma_start(out=ptile, in_=p_t[:, :])

        wq_b = cpool.tile([D, D], BF16)
        wk_b = cpool.tile([D, D], BF16)
        wv_b = cpool.tile([D, D], BF16)
        wp_b = cpool.tile([D, D], BF16)
        nc.vector.tensor_scalar_mul(out=wq_b, in0=wraw[:, 0, :], scalar1=ptile[:, 0:1])
        nc.vector.tensor_scalar(out=wk_b, in0=wraw[:, 1, :],
                                scalar1=ptile[:, 2:3], scalar2=SCALE,
                                op0=OP.mult, op1=OP.mult)
        nc.vector.tensor_scalar_mul(out=wv_b, in0=wraw[:, 2, :], scalar1=ptile[:, 4:5])
        nc.vector.tensor_copy(wp_b, wraw[:, 3, :])

        # bwq = Wq^T bq_ln + bq ; bwv = Wv^T bv_ln + bv ; bpe = bp + Wp^T bwv
        bwq = cpool.tile([D, 1], F32)
        bwv = cpool.tile([D, 1], F32)
        bpe = cpool.tile([D, 1], F32)
        bps = prep.tile([D, 512], F32, tag="prep")
        nc.tensor.matmul(bps[:, 0:1], wraw[:, 0, :], ptile[:, 1:2])
        nc.tensor.matmul(bps[:, 1:2], wraw[:, 2, :], ptile[:, 5:6])
        nc.vector.tensor_add(out=bwq, in0=bps[:, 0:1], in1=ptile[:, 6:7])
        nc.vector.tensor_add(out=bwv, in0=bps[:, 1:2], in1=ptile[:, 8:9])
        bps2 = prep.tile([D, 512], F32, tag="prep")
        nc.tensor.matmul(bps2[:, 0:1], wraw[:, 3, :], bwv[:, 0:1])
        nc.vector.tensor_add(out=bpe, in0=bps2[:, 0:1], in1=ptile[:, 9:10])

        id_bf = cpool.tile([D, D], BF16)
        id_f32 = cpool.tile([D, D], F32)
        make_identity(nc, id_bf)
        make_identity(nc, id_f32)
        ones_bf = cpool.tile([KCH, DIM_HEAD], BF16)
        nc.vector.memset(ones_bf, 1.0)
        eps_c = cpool.tile([D, 1], F32)
        nc.vector.memset(eps_c, EPS)

        # ---------------- per-window pipeline ----------------
        for w in range(NWIN):
            # ---- loads (token index = 2p + c within each view)
            xq = sb.tile([D, NVIEW, 2, D], F32, tag="xq")
            nc.sync.dma_start(
                out=xq,
                in_=q_t[:, w].rearrange("n a b d -> (a b) n d")
                             .rearrange("(p c) n d -> p n (c d)", c=2))
            xk = sb.tile([KCH, 2, D], F32, tag="xk")
            xv = sb.tile([KCH, 2, D], F32, tag="xv")
            for c in range(2):
                nc.sync.dma_start(
                    out=xk[:, c, :],
                    in_=k_t[3 * c:3 * c + 3, w]
                        .rearrange("n a b d -> n (a b) d"))
                nc.sync.dma_start(
                    out=xv[:, c, :],
                    in_=v_t[3 * c:3 * c + 3, w]
                        .rearrange("n a b d -> n (a b) d"))

            # ---- LN stats: groups 0-11 q (n,c), 12-13 k (c), 14-15 v (c)
            st = sb.tile([D, 16, 6], F32, tag="st")
            nc.gpsimd.memset(st[96:, 12:16, :], 1.0)
            for n in range(NVIEW):
                for c in range(2):
                    nc.vector.bn_stats(out=st[:, 2 * n + c, :],
                                       in_=xq[:, n, c, :])
            for c in range(2):
                nc.vector.bn_stats(out=st[:KCH, 12 + c, :], in_=xk[:, c, :])
                nc.vector.bn_stats(out=st[:KCH, 14 + c, :], in_=xv[:, c, :])

            # stats combine on gpsimd (bn_stats gives even/odd halves):
            #  mu = (m_e + m_o)/2 ; var = (v_e + v_o)/128 + (m_e - m_o)^2/4
            # rs = (var+eps)^-1/2 via Ln/Exp with var4 = v_s/32 + d^2,
            # var = var4/4 (Ln scale=0.25).
            sh = sb.tile([D, 16], F32, tag="sh")    # mu
            vs = sb.tile([D, 16], F32, tag="vs")
            dm = sb.tile([D, 16], F32, tag="dm")
            dd = sb.tile([D, 16], F32, tag="dd")
            t32 = sb.tile([D, 16], F32, tag="t32")
            var4 = sb.tile([D, 16], F32, tag="var4")
            nc.gpsimd.tensor_tensor(out=vs, in0=st[:, :, 2], in1=st[:, :, 5], op=OP.add)
            nc.gpsimd.tensor_tensor(out=dm, in0=st[:, :, 1], in1=st[:, :, 4],
                                    op=OP.subtract)
            nc.gpsimd.tensor_tensor(out=dd, in0=dm, in1=dm, op=OP.mult)
            nc.gpsimd.tensor_scalar(out=t32, in0=vs, scalar1=1.0 / 32.0, scalar2=None,
                                    op0=OP.mult)
            nc.gpsimd.tensor_tensor(out=var4, in0=t32, in1=dd, op=OP.add)
            nc.gpsimd.tensor_tensor(out=sh, in0=st[:, :, 1], in1=st[:, :, 4],
                                    op=OP.add)
            nc.gpsimd.tensor_scalar(out=sh, in0=sh, scalar1=0.5, scalar2=None,
                                    op0=OP.mult)

            lnv = sb.tile([D, 16], F32, tag="lnv")
            rs = sb.tile([D, 16], F32, tag="rs")
            nc.scalar.activation(out=lnv, in_=var4, func=AF.Ln,
                                 bias=eps_c[:, 0:1], scale=0.25)
            nc.scalar.activation(out=rs, in_=lnv, func=AF.Exp, scale=-0.5)

            # ---- normalize -> bf16 (all on gpsimd; k/v padded to 128 rows
            # so the DMA transposes below are clean 128x128 tiles)
            xh_q = sb.tile([D, NVIEW, 2, D], BF16, tag="xhq")
            for n in range(NVIEW):
                for c in range(2):
                    j = 2 * n + c
                    nc.vector.tensor_scalar(
                        out=xh_q[:, n, c, :], in0=xq[:, n, c, :],
                        scalar1=sh[:, j:j + 1], scalar2=rs[:, j:j + 1],
                        op0=OP.subtract, op1=OP.mult)
            xh_k = sb.tile([KCH, 2, D], BF16, tag="xhk")
            xh_v = sb.tile([KCH, 2, D], BF16, tag="xhv")
            for c in range(2):
                nc.vector.tensor_scalar(
                    out=xh_k[:KCH, c, :], in0=xk[:, c, :],
                    scalar1=sh[:KCH, 12 + c:13 + c], scalar2=rs[:KCH, 12 + c:13 + c],
                    op0=OP.subtract, op1=OP.mult)
                nc.vector.tensor_scalar(
                    out=xh_v[:KCH, c, :], in0=xv[:, c, :],
                    scalar1=sh[:KCH, 14 + c:15 + c], scalar2=rs[:KCH, 14 + c:15 + c],
                    op0=OP.subtract, op1=OP.mult)

            # ---- transposes to feature-major: PE, then DMA drains (bf16)
            xhqT = sb.tile([D, QTOK], BF16, tag="xhqT")
            xhkvT = sb.tile([D, 4, KCH], BF16, tag="xhkvT")
            tp1 = prep.tile([D, 1024], BF16, tag="prep")
            for j in range(8):
                nc.tensor.transpose(tp1[:, 128 * j:128 * j + 128],
                                    xh_q[:, j // 2, j % 2, :], id_bf)
            nc.vector.tensor_copy(xhqT[:, 0:1024], tp1)
            tp2 = prep.tile([D, 1024], BF16, tag="prep")
            for j in range(4):
                nc.tensor.transpose(tp2[:, 128 * j:128 * j + 128],
                                    xh_q[:, (8 + j) // 2, j % 2, :], id_bf)
            for c in range(2):
                nc.tensor.transpose(tp2[:, 512 + KCH * c:512 + KCH * c + KCH],
                                    xh_k[:, c, :], id_bf[:KCH, :KCH])
                nc.tensor.transpose(tp2[:, 728 + KCH * c:728 + KCH * c + KCH],
                                    xh_v[:, c, :], id_bf[:KCH, :KCH])
            nc.vector.tensor_copy(xhqT[:, 1024:1536], tp2[:, 0:512])
            nc.vector.tensor_copy(xhkvT.rearrange("p g k -> p (g k)"),
                                  tp2[:, 512:944])

            # ---- projections
            qhT = sb.tile([D, QTOK], BF16, tag="qhT")
            for g in range(3):
                pq = prep.tile([D, 512], F32, tag="prep")
                nc.tensor.matmul(pq, wq_b, xhqT[:, 512 * g:512 * g + 512])
                nc.vector.tensor_scalar(
                    out=qhT[:, 512 * g:512 * g + 512], in0=pq,
                    scalar1=bwq[:, 0:1], scalar2=None, op0=OP.add)
            khT = sb.tile([D, 2, KCH], BF16, tag="khT")
            vh = sb.tile([KCH, 2, D], BF16, tag="vh")
            pkv = prep.tile([D, 512], F32, tag="prep")
            for c in range(2):
                nc.tensor.matmul(pkv[:, 128 * c:128 * c + KCH], wk_b,
                                 xhkvT[:, c, :])
                nc.tensor.matmul(pkv[:KCH, 256 + 128 * c:256 + 128 * c + 128],
                                 xhkvT[:, 2 + c, :], wv_b)
            nc.vector.tensor_copy(
                khT, pkv[:, 0:384].rearrange("p (c k) -> p c k", c=3)[:, 0:2, :KCH])
            nc.vector.tensor_copy(
                vh, pkv[:KCH, 256:512].rearrange("p (a f) -> p a f", a=2))

            # ---- attention: per block, per head: scores -> exp; then av/den
            ets = []
            aT = sb.tile([D, QTOK], BF16, tag="aT")
            zps = zpsp.tile([D, 512], F32, tag="zps")
            for hd in range(HEADS):
                et = etp.tile([KCH, 2, QTOK], BF16, tag=f"et{hd}")
                ets.append(et)
            for b in range(NBLK):
                q0 = QB * b
                for hd in range(HEADS):
                    dps = dotp.tile([KCH, 2, QB], F32, tag="dot")
                    for c in range(2):
                        nc.tensor.matmul(
                            dps[:, c, :],
                            khT[32 * hd:32 * hd + 32, c, :],
                            qhT[32 * hd:32 * hd + 32, q0:q0 + QB],
                            tile_position=(32 * hd, 0))
                    nc.scalar.activation(
                        out=ets[hd][:, :, q0:q0 + QB], in_=dps, func=AF.Exp)
                av = avp.tile([D, QB], F32, tag="av")
                den = denp.tile([D, QB], F32, tag="den")
                for hd in range(HEADS):
                    for c in range(2):
                        nc.tensor.matmul(
                            den[32 * hd:32 * hd + 32, :],
                            ones_bf, ets[hd][:, c, q0:q0 + QB],
                            start=(c == 0), stop=(c == 1),
                            tile_position=(0, 32 * hd))
                        nc.tensor.matmul(
                            av[32 * hd:32 * hd + 32, :],
                            vh[:, c, 32 * hd:32 * hd + 32],
                            ets[hd][:, c, q0:q0 + QB],
                            start=(c == 0), stop=(c == 1),
                            tile_position=(0, 32 * hd))
                rln = sb.tile([D, QB], F32, tag="rln")
                recipT = sb.tile([D, QB], F32, tag="recipT")
                nc.scalar.activation(out=rln, in_=den, func=AF.Ln,
                                     bias=eps_c[:, 0:1])
                nc.scalar.activation(out=recipT, in_=rln, func=AF.Exp,
                                     scale=-1.0)
                nc.vector.tensor_tensor(
                    out=aT[:, q0:q0 + QB], in0=av, in1=recipT, op=OP.mult)
                # out-projection: accumulate the two views of this block
                for u in range(2):
                    n = 2 * b + u
                    nc.tensor.matmul(zps[:, 0:256], wp_b,
                                     aT[:, 256 * n:256 * n + 256],
                                     start=(n == 0), stop=(n == NVIEW - 1))

            # ---- epilogue: mean+bias, transpose back, skip, store
            outT = sb.tile([D, 256], F32, tag="outT")
            nc.vector.tensor_scalar(
                out=outT, in0=zps[:, 0:256], scalar1=1.0 / NVIEW,
                scalar2=bpe[:, 0:1], op0=OP.mult, op1=OP.add)
            sk = sb.tile([D, 2, D], F32, tag="sk")
            nc.sync.dma_start(
                out=sk,
                in_=skip_t[w].rearrange("a b d -> (a b) d")
                             .rearrange("(p c) d -> p (c d)", c=2))
            fps = prep.tile([D, 512], F32, tag="prep")
            for i in range(2):
                nc.tensor.transpose(fps[:, 128 * i:128 * i + 128],
                                    outT[:, 128 * i:128 * i + 128], id_f32)
            res = sb.tile([D, 2, D], F32, tag="res")
            nc.vector.tensor_tensor(
                out=res, in0=fps[:, 0:256].rearrange("p (c d) -> p c d", c=2),
                in1=sk, op=OP.add)
            nc.sync.dma_start(
                out=out_t[w].rearrange("a b d -> (a b) d")
                            .rearrange("(p c) d -> p (c d)", c=2),
                in_=res)

    _split_waits(nc)
    return nc


_NC_CACHE = None


def _get_nc():
    global _NC_CACHE
    if _NC_CACHE is None:
        _NC_CACHE = build_nc()
    return _NC_CACHE


def kernel(**inputs):
    q = np.asarray(inputs["q"], dtype=np.float32)
    k = np.asarray(inputs["k"], dtype=np.float32)
    v = np.asarray(inputs["v"], dtype=np.float32)
    skip = np.asarray(inputs["skip"], dtype=np.float32)

    wstack = np.stack([inputs["Wq"], inputs["Wk"], inputs["Wv"], inputs["Wp"]]
                      ).astype(np.float32)
    pstack = np.stack([
        inputs["gq"], inputs["bq_ln"], inputs["gk"], inputs["bk_ln"],
        inputs["gv"], inputs["bv_ln"], inputs["bq"], inputs["bk"],
        inputs["bv"], inputs["bp"]], axis=1).astype(np.float32)

    nc = _get_nc()
    in_maps = []
    for c in range(8):
        in_maps.append({
            "q": np.ascontiguousarray(q[0, :, c]),
            "k": np.ascontiguousarray(k[0, :, c]),
            "v": np.ascontiguousarray(v[0, :, c]),
            "skip": np.ascontiguousarray(skip[0, c]),
            "wstack": wstack,
            "pstack": pstack,
        })
    import os
    trace = bool(os.environ.get("KERNEL_TRACE"))
    res = run_bass_kernel_spmd(nc, in_maps, core_ids=list(range(8)),
                               trace=trace)
    kernel.last_result = res
    out = np.stack([res.results[c]["out"] for c in range(8)], axis=0)
    return out[None]  # (1, 8, 8, 16, 16, 128)
